# revision 22
# baseline (speedup 1.0000x reference)
"""MoE (2-expert SwiGLU) Trainium2 kernel, 8-core SPMD.

Strategy: since the MLPs have no biases and silu(0) = 0, MLP(0) = 0, so each
token only needs the expert it is routed to.  The host gathers tokens by
expert (MoE dispatch), cores 0-3 process expert-0 tokens and cores 4-7
expert-1 tokens (~1/8 of total tokens per core), each core running a dense
SwiGLU MLP with its expert's weights.  The host scatters per-core outputs
back into the full [B, S, D] output.  This halves FLOPs vs. the reference's
dense-masked formulation and needs no collectives.

Device dataflow (per core, transposed so no on-chip transposes are needed):
  yT = Wd^T @ (silu(Wg^T @ xT) * (Wu^T @ xT))
Weights are the stationary matmul operand, token-columns the moving operand.
All matmuls are bf16 with fp32 PSUM accumulation.  The FF intermediate `h`
for all of a core's tokens stays resident in SBUF, so each weight byte is
DMA'd exactly once per core.

Schedule notes (from trace analysis):
  - Startup DMAs are interleaved across the two HWDGE queues (sync +
    scalar) in k-slice consumption order, weights woven just-in-time
    between x slices; the two queues share ~370 B/ns of HBM bandwidth and
    each drains in issue order, so nothing bulky may sit ahead of an
    early-needed slice.
  - A dummy-matmul burst on zeroed scratch keeps the PE busy from program
    start until the startup DMAs land, so the HAM clock-gate opens during
    the warm-up and the real matmul stream starts at full clock.
  - Stage 2 for token-tiles 0/1 of the first d_model block runs in the two
    PSUM banks stage 1 doesn't use, while stage 1's last PSUM consumers
    drain -- no PE gap at the stage boundary.
  - The last d_model block's PSUM->SBUF copies are spread over vector and
    scalar engines and its stores over both DMA queues, shortening the
    tail chain after the final matmul.
"""

import sys

for _p in ("/opt/trn_rl_repo", "/root/.axon_site/_ro/trn_rl_repo"):
    if _p not in sys.path:
        sys.path.append(_p)

import numpy as np
import ml_dtypes

BF16 = ml_dtypes.bfloat16

D_MODEL = 1024
D_FF = 4096
P = 128
KD = D_MODEL // P  # 8   k-tiles over d_model
MF = D_FF // P     # 32  tiles over d_ff
N_CORES = 8
CPE = 4            # cores per expert
NT = 3             # token tiles per core

_program_cache: dict[tuple, object] = {}


def _token_tiles(maxpc: int) -> tuple:
    """Split the per-core token capacity into 3 near-equal tiles, the first
    two multiples of 4 (keeps every column offset 8-byte aligned in bf16),
    each <= 512."""
    C = max(maxpc, 24)
    t = 4 * ((C + 3 * 4 - 1) // (3 * 4))
    tiles = (t, t, C - 2 * t)
    assert all(0 < s <= 512 for s in tiles), (maxpc, tiles)
    return tiles


def _build_program(tiles: tuple):
    """Bass program for one core: x [D,C] -> y [D,C], C = sum(tiles) tokens."""
    import concourse.tile as tile
    from concourse import mybir, bacc

    C = sum(tiles)
    offs = [sum(tiles[:i]) for i in range(len(tiles))]
    TSMAX = max(tiles)
    f32 = mybir.dt.float32
    b16 = mybir.dt.bfloat16

    nc = bacc.Bacc()
    xT = nc.declare_dram_parameter("xT", [P, KD, C], b16, isOutput=False)
    # w1[mf, p, gu, kd, c] = (wg if gu==0 else wu)[kd*128 + p, mf*128 + c]
    w1 = nc.declare_dram_parameter("w1", [MF, P, 2, KD, P], b16, isOutput=False)
    # wdp[md, p, kf, c] = wd[kf*128 + p, md*128 + c]
    wdp = nc.declare_dram_parameter("wd", [KD, P, MF, P], b16, isOutput=False)
    yT = nc.declare_dram_parameter("yT", [KD, P, C], b16, isOutput=True)

    with tile.TileContext(nc) as tc:
        with tc.tile_pool(name="sb", bufs=1) as sb:
            wp = wk = sb
            # PE warm-up scratch (zeroed so CoreSim sees no uninit reads)
            zw = sb.tile([P, P], b16, tag="zw", name="zw")
            zx = sb.tile([P, 256], b16, tag="zx", name="zx")
            nc.gpsimd.memset(zw[:], 0)
            nc.vector.memset(zx[:], 0)

            x_sb = sb.tile([P, KD, C], b16, tag="x", name="x_sb")
            h_sb = sb.tile([P, MF, C], b16, tag="h", name="h_sb")
            wt0 = wp.tile([P, 2, KD, P], b16, tag="wt", bufs=3, name="wt_0")

            # Startup DMAs interleaved across the two HWDGE queues (each
            # issue costs ~0.6us serialized on its queue engine, and each
            # queue's transfers drain in issue order, so neither queue may
            # carry the whole ramp and nothing big may sit ahead of an
            # early-needed slice).  Weight k-slices kd0/kd1 ride the scalar
            # queue; x k-slices alternate, with the bulky kd2-7 weight chunk
            # on sync only after x1.
            nc.scalar.dma_start(wt0[:, :, :2], w1[0, :, :, :2])   # kd0-1, 128 KB
            nc.sync.dma_start(x_sb[:, 0], xT[:, 0])
            nc.scalar.dma_start(x_sb[:, 1], xT[:, 1])
            nc.sync.dma_start(wt0[:, :, 2:4], w1[0, :, :, 2:4])   # kd2-3
            nc.scalar.dma_start(x_sb[:, 2], xT[:, 2])
            nc.sync.dma_start(x_sb[:, 3], xT[:, 3])
            nc.scalar.dma_start(x_sb[:, 4], xT[:, 4])
            nc.sync.dma_start(wt0[:, :, 4:6], w1[0, :, :, 4:6])   # kd4-5
            nc.scalar.dma_start(wt0[:, :, 6:], w1[0, :, :, 6:])   # kd6-7
            nc.sync.dma_start(x_sb[:, 5], xT[:, 5])
            nc.scalar.dma_start(x_sb[:, 6], xT[:, 6])
            nc.sync.dma_start(x_sb[:, 7], xT[:, 7])

            # Dummy matmul burst: keeps the PE continuously busy from ~t0
            # until the startup DMAs have landed (~4.5us), so the HAM
            # clock-gate opens DURING the warm-up and the real matmul stream
            # starts at full clock with no stalls.  One accumulation group
            # -> no inter-matmul semaphores.
            with tc.tile_pool(name="ps0", bufs=1, space="PSUM") as ps0:
                pz = ps0.tile([P, 256], f32, tag="pz", name="pz")
                NWARM = 16
                for i in range(NWARM):
                    nc.tensor.matmul(pz[:], zw[:], zx[:],
                                     start=(i == 0), stop=(i == NWARM - 1))

            wdt = [None, None]

            # Stage 1: h = silu(Wg^T x) * (Wu^T x), laid out [ff-part, C]
            with tc.tile_pool(name="ps1", bufs=NT, space="PSUM") as ps1:
                for mf in range(MF):
                    if mf == 0:
                        wt = wt0
                    else:
                        wt = wp.tile([P, 2, KD, P], b16, tag="wt", bufs=3,
                                     name=f"wt_{mf}")
                        nc.sync.dma_start(wt[:], w1[mf])
                    if mf in (6, 10):
                        i = 0 if mf == 6 else 1
                        wdt[i] = wp.tile([P, MF, P], b16, tag="wdt", bufs=2,
                                         name=f"wdt_{i}")
                        nc.sync.dma_start(wdt[i][:], wdp[i])
                    psg = [ps1.tile([P, 512], f32, tag="psg", name=f"psg_{mf}_{t}")
                           for t in range(NT)]
                    psu = [ps1.tile([P, 512], f32, tag="psu", name=f"psu_{mf}_{t}")
                           for t in range(NT)]
                    for kd in range(KD):
                        for gu in range(2):
                            ps = psg if gu == 0 else psu
                            for t in range(NT):
                                nc.tensor.matmul(
                                    ps[t][:, :tiles[t]],
                                    wt[:, gu, kd],
                                    x_sb[:, kd, offs[t]:offs[t] + tiles[t]],
                                    start=(kd == 0),
                                    stop=(kd == KD - 1),
                                )
                    for t in range(NT):
                        sil = wk.tile([P, TSMAX], f32, tag="sil", bufs=4,
                                      name=f"sil_{mf}_{t}")
                        nc.scalar.activation(
                            sil[:, :tiles[t]], psg[t][:, :tiles[t]],
                            mybir.ActivationFunctionType.Silu,
                        )
                        nc.vector.tensor_mul(
                            h_sb[:, mf, offs[t]:offs[t] + tiles[t]],
                            sil[:, :tiles[t]], psu[t][:, :tiles[t]],
                        )

                # Stage 2, md=0, token tiles 0/1: runs in the two PSUM banks
                # ps1 doesn't use, so the PE rolls straight from stage 1 into
                # these while stage 1's last silu/mul drain ps1.
                with tc.tile_pool(name="ps2a", bufs=2, space="PSUM") as ps2a:
                    y_md0 = wk.tile([P, C], b16, tag="y", bufs=2, name="y_0")
                    psyA = [ps2a.tile([P, 512], f32, tag="psyA",
                                      name=f"psyA_{t}") for t in range(2)]
                    for kf in range(MF):
                        for t in range(2):
                            nc.tensor.matmul(
                                psyA[t][:, :tiles[t]],
                                wdt[0][:, kf],
                                h_sb[:, kf, offs[t]:offs[t] + tiles[t]],
                                start=(kf == 0),
                                stop=(kf == MF - 1),
                            )
                    for t in range(2):
                        nc.vector.tensor_copy(
                            y_md0[:, offs[t]:offs[t] + tiles[t]],
                            psyA[t][:, :tiles[t]],
                        )
                        nc.scalar.dma_start(
                            yT[0, :, offs[t]:offs[t] + tiles[t]],
                            y_md0[:, offs[t]:offs[t] + tiles[t]],
                        )

            # Stage 2 remainder: y = Wd^T h, laid out [d-part, C]
            with tc.tile_pool(name="ps2", bufs=6, space="PSUM") as ps2:
                # md=0 token tile 2 (weights still resident in wdt[0])
                psy0 = ps2.tile([P, 512], f32, tag="psy", name="psy_0_2")
                for kf in range(MF):
                    nc.tensor.matmul(
                        psy0[:, :tiles[2]],
                        wdt[0][:, kf],
                        h_sb[:, kf, offs[2]:offs[2] + tiles[2]],
                        start=(kf == 0),
                        stop=(kf == MF - 1),
                    )
                nc.vector.tensor_copy(
                    y_md0[:, offs[2]:offs[2] + tiles[2]],
                    psy0[:, :tiles[2]],
                )
                nc.scalar.dma_start(
                    yT[0, :, offs[2]:offs[2] + tiles[2]],
                    y_md0[:, offs[2]:offs[2] + tiles[2]],
                )

                for md in range(1, KD):
                    if md == 1:
                        wdt_md = wdt[1]
                    else:
                        wdt_md = wp.tile([P, MF, P], b16, tag="wdt", bufs=2,
                                         name=f"wdt_{md}")
                        nc.sync.dma_start(wdt_md[:], wdp[md])
                    y_sb = wk.tile([P, C], b16, tag="y", bufs=2,
                                   name=f"y_{md}")
                    pieces = [(offs[t], tiles[t]) for t in range(NT)]
                    psy = [ps2.tile([P, 512], f32, tag="psy",
                                    name=f"psy_{md}_{i}")
                           for i in range(len(pieces))]
                    for kf in range(MF):
                        for i, (o, n) in enumerate(pieces):
                            nc.tensor.matmul(
                                psy[i][:, :n],
                                wdt_md[:, kf],
                                h_sb[:, kf, o:o + n],
                                start=(kf == 0),
                                stop=(kf == MF - 1),
                            )
                    last_md = md == KD - 1
                    for i, (o, n) in enumerate(pieces):
                        # For the final block, spread the PSUM->SBUF copies
                        # over vector+scalar and the stores over both DMA
                        # queues so the tail chain is as short as possible.
                        if last_md and i == 1:
                            nc.scalar.copy(y_sb[:, o:o + n], psy[i][:, :n])
                        else:
                            nc.vector.tensor_copy(
                                y_sb[:, o:o + n], psy[i][:, :n],
                            )
                        q = nc.sync if (last_md and i % 2 == 0) else nc.scalar
                        q.dma_start(
                            yT[md, :, o:o + n], y_sb[:, o:o + n],
                        )

    nc.compile()
    return nc


def _pack_w1(wg: np.ndarray, wu: np.ndarray) -> np.ndarray:
    """[D, F] x2 -> [MF, P, 2, KD, P] bf16, matching the kernel's layout."""
    # w1[mf, p, gu, kd, c] = w_gu[kd*128 + p, mf*128 + c]
    stack = np.stack([wg, wu], axis=0)            # [2, D, F]
    r = stack.reshape(2, KD, P, MF, P)            # [gu, kd, p, mf, c]
    return np.ascontiguousarray(r.transpose(3, 2, 0, 1, 4)).astype(BF16)


def _pack_wd(wd: np.ndarray) -> np.ndarray:
    """[F, D] -> [KD, P, MF, P] bf16. wdp[md, p, kf, c] = wd[kf*128+p, md*128+c]"""
    r = wd.reshape(MF, P, KD, P)                  # [kf, p, md, c]
    return np.ascontiguousarray(r.transpose(2, 1, 0, 3)).astype(BF16)


def _run_device(in_maps, tiles):
    from concourse.bass_utils import run_bass_kernel_spmd

    if tiles not in _program_cache:
        _program_cache[tiles] = _build_program(tiles)
    nc = _program_cache[tiles]
    res = run_bass_kernel_spmd(nc, in_maps, core_ids=list(range(N_CORES)))
    return [r["yT"] for r in res.results]


def kernel(hidden_states, routing_mask, wg0, wu0, wd0, wg1, wu1, wd1,
           _run=None):
    hidden_states = np.asarray(hidden_states, dtype=np.float32)
    routing_mask = np.asarray(routing_mask)
    B, S, D = hidden_states.shape
    NTOK = B * S
    x = hidden_states.reshape(NTOK, D)
    mask = routing_mask.reshape(NTOK)

    idx = [np.nonzero(mask == e)[0] for e in (0, 1)]
    maxpc = max(
        (len(idx[0]) + CPE - 1) // CPE,
        (len(idx[1]) + CPE - 1) // CPE,
        1,
    )
    tiles = _token_tiles(maxpc)
    C = sum(tiles)

    w1_packed = [_pack_w1(np.asarray(wg0), np.asarray(wu0)),
                 _pack_w1(np.asarray(wg1), np.asarray(wu1))]
    wd_packed = [_pack_wd(np.asarray(wd0)), _pack_wd(np.asarray(wd1))]

    in_maps = []
    chunks = []  # (expert, token_indices) per core
    for core in range(N_CORES):
        e = core // CPE
        slot = core % CPE
        ids = idx[e]
        # split ids into CPE nearly-equal chunks
        bounds = [(len(ids) * i) // CPE for i in range(CPE + 1)]
        ids_c = ids[bounds[slot]:bounds[slot + 1]]
        chunks.append((e, ids_c))

        xc = np.zeros((C, D), dtype=np.float32)
        xc[: len(ids_c)] = x[ids_c]
        # xT[p, kd, c] = xc[c, kd*128 + p]
        xT = np.ascontiguousarray(
            xc.reshape(C, KD, P).transpose(2, 1, 0)
        ).astype(BF16)
        in_maps.append({
            "xT": xT,
            "w1": w1_packed[e],
            "wd": wd_packed[e],
        })

    run = _run if _run is not None else _run_device
    outs = run(in_maps, tiles)

    y_full = np.zeros((NTOK, D), dtype=np.float32)
    for core in range(N_CORES):
        _, ids_c = chunks[core]
        if len(ids_c) == 0:
            continue
        yT = np.asarray(outs[core]).astype(np.float32).reshape(D, C)
        y_full[ids_c] = yT[:, : len(ids_c)].T
    return y_full.reshape(B, S, D)


# revision 23
# speedup vs baseline: 1.0005x; 1.0005x over previous
"""MoE (2-expert SwiGLU) Trainium2 kernel, 8-core SPMD.

Strategy: since the MLPs have no biases and silu(0) = 0, MLP(0) = 0, so each
token only needs the expert it is routed to.  The host gathers tokens by
expert (MoE dispatch), cores 0-3 process expert-0 tokens and cores 4-7
expert-1 tokens (~1/8 of total tokens per core), each core running a dense
SwiGLU MLP with its expert's weights.  The host scatters per-core outputs
back into the full [B, S, D] output.  This halves FLOPs vs. the reference's
dense-masked formulation and needs no collectives.

Device dataflow (per core, transposed so no on-chip transposes are needed):
  yT = Wd^T @ (silu(Wg^T @ xT) * (Wu^T @ xT))
Weights are the stationary matmul operand, token-columns the moving operand.
All matmuls are bf16 with fp32 PSUM accumulation.  The FF intermediate `h`
for all of a core's tokens stays resident in SBUF, so each weight byte is
DMA'd exactly once per core.

Schedule notes (from trace analysis):
  - Startup DMAs are interleaved across the two HWDGE queues (sync +
    scalar) in k-slice consumption order, weights woven just-in-time
    between x slices; the two queues share ~370 B/ns of HBM bandwidth and
    each drains in issue order, so nothing bulky may sit ahead of an
    early-needed slice.
  - A dummy-matmul burst on zeroed scratch keeps the PE busy from program
    start until the startup DMAs land, so the HAM clock-gate opens during
    the warm-up and the real matmul stream starts at full clock.
  - Stage 2 for token-tiles 0/1 of the first d_model block runs in the two
    PSUM banks stage 1 doesn't use, while stage 1's last PSUM consumers
    drain -- no PE gap at the stage boundary.
  - The last d_model block's PSUM->SBUF copies are spread over vector and
    scalar engines and its stores over both DMA queues, shortening the
    tail chain after the final matmul.
"""

import sys

for _p in ("/opt/trn_rl_repo", "/root/.axon_site/_ro/trn_rl_repo"):
    if _p not in sys.path:
        sys.path.append(_p)

import numpy as np
import ml_dtypes

BF16 = ml_dtypes.bfloat16

D_MODEL = 1024
D_FF = 4096
P = 128
KD = D_MODEL // P  # 8   k-tiles over d_model
MF = D_FF // P     # 32  tiles over d_ff
N_CORES = 8
CPE = 4            # cores per expert
NT = 3             # token tiles per core

_program_cache: dict[tuple, object] = {}


def _token_tiles(maxpc: int) -> tuple:
    """Split the per-core token capacity into 3 near-equal tiles, the first
    two multiples of 4 (keeps every column offset 8-byte aligned in bf16),
    each <= 512."""
    C = max(maxpc, 24)
    t = 4 * ((C + 3 * 4 - 1) // (3 * 4))
    tiles = (t, t, C - 2 * t)
    assert all(0 < s <= 512 for s in tiles), (maxpc, tiles)
    return tiles


def _build_program(tiles: tuple):
    """Bass program for one core: x [D,C] -> y [D,C], C = sum(tiles) tokens."""
    import concourse.tile as tile
    from concourse import mybir, bacc

    C = sum(tiles)
    offs = [sum(tiles[:i]) for i in range(len(tiles))]
    TSMAX = max(tiles)
    f32 = mybir.dt.float32
    b16 = mybir.dt.bfloat16

    nc = bacc.Bacc()
    xT = nc.declare_dram_parameter("xT", [P, KD, C], b16, isOutput=False)
    # w1[mf, p, gu, kd, c] = (wg if gu==0 else wu)[kd*128 + p, mf*128 + c]
    w1 = nc.declare_dram_parameter("w1", [MF, P, 2, KD, P], b16, isOutput=False)
    # wdp[md, p, kf, c] = wd[kf*128 + p, md*128 + c]
    wdp = nc.declare_dram_parameter("wd", [KD, P, MF, P], b16, isOutput=False)
    yT = nc.declare_dram_parameter("yT", [KD, P, C], b16, isOutput=True)

    with tile.TileContext(nc) as tc:
        with tc.tile_pool(name="sb", bufs=1) as sb:
            wp = wk = sb
            # PE warm-up scratch (zeroed so CoreSim sees no uninit reads)
            zw = sb.tile([P, P], b16, tag="zw", name="zw")
            zx = sb.tile([P, 256], b16, tag="zx", name="zx")
            nc.gpsimd.memset(zw[:], 0)
            nc.vector.memset(zx[:], 0)

            x_sb = sb.tile([P, KD, C], b16, tag="x", name="x_sb")
            h_sb = sb.tile([P, MF, C], b16, tag="h", name="h_sb")
            wt0 = wp.tile([P, 2, KD, P], b16, tag="wt", bufs=3, name="wt_0")

            # Startup DMAs interleaved across the two HWDGE queues (each
            # issue costs ~0.6us serialized on its queue engine, and each
            # queue's transfers drain in issue order, so neither queue may
            # carry the whole ramp and nothing big may sit ahead of an
            # early-needed slice).  x k-slices alternate between the queues
            # in consumption order, with 2-slice weight chunks woven in
            # just-in-time ahead of the k-slices that need them.
            nc.scalar.dma_start(wt0[:, :, :2], w1[0, :, :, :2])   # kd0-1, 128 KB
            nc.sync.dma_start(x_sb[:, 0], xT[:, 0])
            nc.scalar.dma_start(x_sb[:, 1], xT[:, 1])
            nc.sync.dma_start(wt0[:, :, 2:4], w1[0, :, :, 2:4])   # kd2-3
            nc.scalar.dma_start(x_sb[:, 2], xT[:, 2])
            nc.sync.dma_start(x_sb[:, 3], xT[:, 3])
            nc.scalar.dma_start(x_sb[:, 4], xT[:, 4])
            nc.sync.dma_start(wt0[:, :, 4:6], w1[0, :, :, 4:6])   # kd4-5
            nc.scalar.dma_start(wt0[:, :, 6:], w1[0, :, :, 6:])   # kd6-7
            nc.sync.dma_start(x_sb[:, 5], xT[:, 5])
            nc.scalar.dma_start(x_sb[:, 6], xT[:, 6])
            nc.sync.dma_start(x_sb[:, 7], xT[:, 7])

            # Dummy matmul burst: keeps the PE continuously busy from ~t0
            # until the startup DMAs have landed (~4.5us), so the HAM
            # clock-gate opens DURING the warm-up and the real matmul stream
            # starts at full clock with no stalls.  One accumulation group
            # -> no inter-matmul semaphores.
            with tc.tile_pool(name="ps0", bufs=1, space="PSUM") as ps0:
                pz = ps0.tile([P, 256], f32, tag="pz", name="pz")
                NWARM = 16
                for i in range(NWARM):
                    nc.tensor.matmul(pz[:], zw[:], zx[:],
                                     start=(i == 0), stop=(i == NWARM - 1))

            wdt = [None, None]

            # Stage 1: h = silu(Wg^T x) * (Wu^T x), laid out [ff-part, C]
            with tc.tile_pool(name="ps1", bufs=NT, space="PSUM") as ps1:
                for mf in range(MF):
                    if mf == 0:
                        wt = wt0
                    else:
                        wt = wp.tile([P, 2, KD, P], b16, tag="wt", bufs=3,
                                     name=f"wt_{mf}")
                        nc.sync.dma_start(wt[:], w1[mf])
                    if mf in (6, 10):
                        i = 0 if mf == 6 else 1
                        wdt[i] = wp.tile([P, MF, P], b16, tag="wdt", bufs=2,
                                         name=f"wdt_{i}")
                        nc.sync.dma_start(wdt[i][:], wdp[i])
                    psg = [ps1.tile([P, 512], f32, tag="psg", name=f"psg_{mf}_{t}")
                           for t in range(NT)]
                    psu = [ps1.tile([P, 512], f32, tag="psu", name=f"psu_{mf}_{t}")
                           for t in range(NT)]
                    for kd in range(KD):
                        for gu in range(2):
                            ps = psg if gu == 0 else psu
                            for t in range(NT):
                                nc.tensor.matmul(
                                    ps[t][:, :tiles[t]],
                                    wt[:, gu, kd],
                                    x_sb[:, kd, offs[t]:offs[t] + tiles[t]],
                                    start=(kd == 0),
                                    stop=(kd == KD - 1),
                                )
                    for t in range(NT):
                        sil = wk.tile([P, TSMAX], f32, tag="sil", bufs=4,
                                      name=f"sil_{mf}_{t}")
                        nc.scalar.activation(
                            sil[:, :tiles[t]], psg[t][:, :tiles[t]],
                            mybir.ActivationFunctionType.Silu,
                        )
                        nc.vector.tensor_mul(
                            h_sb[:, mf, offs[t]:offs[t] + tiles[t]],
                            sil[:, :tiles[t]], psu[t][:, :tiles[t]],
                        )

                # Stage 2, md=0, token tiles 0/1: runs in the two PSUM banks
                # ps1 doesn't use, so the PE rolls straight from stage 1 into
                # these while stage 1's last silu/mul drain ps1.
                with tc.tile_pool(name="ps2a", bufs=2, space="PSUM") as ps2a:
                    y_md0 = wk.tile([P, C], b16, tag="y", bufs=2, name="y_0")
                    psyA = [ps2a.tile([P, 512], f32, tag="psyA",
                                      name=f"psyA_{t}") for t in range(2)]
                    for kf in range(MF):
                        for t in range(2):
                            nc.tensor.matmul(
                                psyA[t][:, :tiles[t]],
                                wdt[0][:, kf],
                                h_sb[:, kf, offs[t]:offs[t] + tiles[t]],
                                start=(kf == 0),
                                stop=(kf == MF - 1),
                            )
                    for t in range(2):
                        nc.vector.tensor_copy(
                            y_md0[:, offs[t]:offs[t] + tiles[t]],
                            psyA[t][:, :tiles[t]],
                        )
                        nc.scalar.dma_start(
                            yT[0, :, offs[t]:offs[t] + tiles[t]],
                            y_md0[:, offs[t]:offs[t] + tiles[t]],
                        )

            # Stage 2 remainder: y = Wd^T h, laid out [d-part, C]
            with tc.tile_pool(name="ps2", bufs=6, space="PSUM") as ps2:
                # md=0 token tile 2 (weights still resident in wdt[0])
                psy0 = ps2.tile([P, 512], f32, tag="psy", name="psy_0_2")
                for kf in range(MF):
                    nc.tensor.matmul(
                        psy0[:, :tiles[2]],
                        wdt[0][:, kf],
                        h_sb[:, kf, offs[2]:offs[2] + tiles[2]],
                        start=(kf == 0),
                        stop=(kf == MF - 1),
                    )
                nc.vector.tensor_copy(
                    y_md0[:, offs[2]:offs[2] + tiles[2]],
                    psy0[:, :tiles[2]],
                )
                nc.scalar.dma_start(
                    yT[0, :, offs[2]:offs[2] + tiles[2]],
                    y_md0[:, offs[2]:offs[2] + tiles[2]],
                )

                for md in range(1, KD):
                    if md == 1:
                        wdt_md = wdt[1]
                    else:
                        wdt_md = wp.tile([P, MF, P], b16, tag="wdt", bufs=2,
                                         name=f"wdt_{md}")
                        nc.sync.dma_start(wdt_md[:], wdp[md])
                    y_sb = wk.tile([P, C], b16, tag="y", bufs=2,
                                   name=f"y_{md}")
                    pieces = [(offs[t], tiles[t]) for t in range(NT)]
                    psy = [ps2.tile([P, 512], f32, tag="psy",
                                    name=f"psy_{md}_{i}")
                           for i in range(len(pieces))]
                    for kf in range(MF):
                        for i, (o, n) in enumerate(pieces):
                            nc.tensor.matmul(
                                psy[i][:, :n],
                                wdt_md[:, kf],
                                h_sb[:, kf, o:o + n],
                                start=(kf == 0),
                                stop=(kf == MF - 1),
                            )
                    last_md = md == KD - 1
                    for i, (o, n) in enumerate(pieces):
                        # For the final block, spread the PSUM->SBUF copies
                        # over vector+scalar and the stores over both DMA
                        # queues so the tail chain is as short as possible.
                        if last_md and i == 1:
                            nc.scalar.copy(y_sb[:, o:o + n], psy[i][:, :n])
                        else:
                            nc.vector.tensor_copy(
                                y_sb[:, o:o + n], psy[i][:, :n],
                            )
                        q = nc.sync if (last_md and i % 2 == 0) else nc.scalar
                        q.dma_start(
                            yT[md, :, o:o + n], y_sb[:, o:o + n],
                        )

    nc.compile()
    return nc


def _pack_w1(wg: np.ndarray, wu: np.ndarray) -> np.ndarray:
    """[D, F] x2 -> [MF, P, 2, KD, P] bf16, matching the kernel's layout."""
    # w1[mf, p, gu, kd, c] = w_gu[kd*128 + p, mf*128 + c]
    stack = np.stack([wg, wu], axis=0)            # [2, D, F]
    r = stack.reshape(2, KD, P, MF, P)            # [gu, kd, p, mf, c]
    return np.ascontiguousarray(r.transpose(3, 2, 0, 1, 4)).astype(BF16)


def _pack_wd(wd: np.ndarray) -> np.ndarray:
    """[F, D] -> [KD, P, MF, P] bf16. wdp[md, p, kf, c] = wd[kf*128+p, md*128+c]"""
    r = wd.reshape(MF, P, KD, P)                  # [kf, p, md, c]
    return np.ascontiguousarray(r.transpose(2, 1, 0, 3)).astype(BF16)


def _run_device(in_maps, tiles):
    from concourse.bass_utils import run_bass_kernel_spmd

    if tiles not in _program_cache:
        _program_cache[tiles] = _build_program(tiles)
    nc = _program_cache[tiles]
    res = run_bass_kernel_spmd(nc, in_maps, core_ids=list(range(N_CORES)))
    return [r["yT"] for r in res.results]


def kernel(hidden_states, routing_mask, wg0, wu0, wd0, wg1, wu1, wd1,
           _run=None):
    hidden_states = np.asarray(hidden_states, dtype=np.float32)
    routing_mask = np.asarray(routing_mask)
    B, S, D = hidden_states.shape
    NTOK = B * S
    x = hidden_states.reshape(NTOK, D)
    mask = routing_mask.reshape(NTOK)

    idx = [np.nonzero(mask == e)[0] for e in (0, 1)]
    maxpc = max(
        (len(idx[0]) + CPE - 1) // CPE,
        (len(idx[1]) + CPE - 1) // CPE,
        1,
    )
    tiles = _token_tiles(maxpc)
    C = sum(tiles)

    w1_packed = [_pack_w1(np.asarray(wg0), np.asarray(wu0)),
                 _pack_w1(np.asarray(wg1), np.asarray(wu1))]
    wd_packed = [_pack_wd(np.asarray(wd0)), _pack_wd(np.asarray(wd1))]

    in_maps = []
    chunks = []  # (expert, token_indices) per core
    for core in range(N_CORES):
        e = core // CPE
        slot = core % CPE
        ids = idx[e]
        # split ids into CPE nearly-equal chunks
        bounds = [(len(ids) * i) // CPE for i in range(CPE + 1)]
        ids_c = ids[bounds[slot]:bounds[slot + 1]]
        chunks.append((e, ids_c))

        xc = np.zeros((C, D), dtype=np.float32)
        xc[: len(ids_c)] = x[ids_c]
        # xT[p, kd, c] = xc[c, kd*128 + p]
        xT = np.ascontiguousarray(
            xc.reshape(C, KD, P).transpose(2, 1, 0)
        ).astype(BF16)
        in_maps.append({
            "xT": xT,
            "w1": w1_packed[e],
            "wd": wd_packed[e],
        })

    run = _run if _run is not None else _run_device
    outs = run(in_maps, tiles)

    y_full = np.zeros((NTOK, D), dtype=np.float32)
    for core in range(N_CORES):
        _, ids_c = chunks[core]
        if len(ids_c) == 0:
            continue
        yT = np.asarray(outs[core]).astype(np.float32).reshape(D, C)
        y_full[ids_c] = yT[:, : len(ids_c)].T
    return y_full.reshape(B, S, D)
